# revision 36
# baseline (speedup 1.0000x reference)
"""Trainium2 Bass kernel for CustomSchNet (nn_CustomSchNet_43456479101225).

Strategy (graph-level data parallel, 8 cores):
  - 512 molecules load-balanced into 128 windows of 4 molecules (128 atoms),
    16 windows per core.
  - The radius graph is symmetric and the filter W depends only on the
    distance, so the edge-filter MLP + ShiftedSoftplus run once per
    UNDIRECTED edge; each W tile feeds the two directed messages.
  - Filter MLP runs with channels on partitions / edges on free dim
    (weights stationary); the flip to edges-on-partitions is free by using
    the ssp output as the stationary operand of the second MLP matmul.
  - Gather/scatter are one-hot matmuls against host-built fp16/fp8 slabs
    (cosine cutoff folded into the gather slab); messages are fp16 2x-mode
    DVE multiplies.
  - ShiftedSoftplus = Ln(1 + Exp(x)); -ln(2) shifts folded into downstream
    biases on host. Emission order is software-pipelined (4 stages) so each
    cross-engine dependency has superchunks of slack.
"""

import math
import numpy as np

HID = 128
NG = 50
CUT = 6.0
L_INT = 3
APM = 32
N_MOL = 512
N_ATOMS = N_MOL * APM
N_CORES = 8
WPC = 16            # windows per core
APW = 128           # atoms per window (4 molecules)
MPW = 4             # molecules per window
LN2 = math.log(2.0)

_PROG_CACHE = {}


# ----------------------------------------------------------------------------
# Device program
# ----------------------------------------------------------------------------
def _build_program(EU):
    """Build + compile the per-core program. EU = padded undirected
    edges/window (multiple of 128); directed EW = 2*EU."""
    import concourse.bacc as bacc
    import concourse.tile as tile
    import concourse.mybir as mybir
    from contextlib import ExitStack

    # Exp and Ln live in different activation-function tables by default, so
    # the table-load pass would ping-pong 1.3us loads between ssp passes.
    # Reorder so the table holding BOTH (natural_log_exp_and_others) wins.
    import concourse.hw_specs as hw_specs
    if not getattr(hw_specs, "_schnet_tbl_patch", False):
        _orig_gat = hw_specs.get_activation_tables

        def _gat(arch):
            t = dict(_orig_gat(arch))
            key = "natural_log_exp_and_others"
            if key in t:
                t = {key: t[key], **{k: v for k, v in t.items() if k != key}}
            return t

        hw_specs._schnet_tbl_patch = True
        bacc.get_activation_tables = _gat

    F32 = mybir.dt.float32
    F16 = mybir.dt.float16
    F8 = mybir.dt.float8e4
    AF = mybir.ActivationFunctionType
    ALU = mybir.AluOpType
    AX = mybir.AxisListType

    EW = 2 * EU
    NCHU = EU // 128                 # undirected chunks per window
    NCHD = 2 * NCHU                  # directed chunks per window
    # undirected superchunks (<=4 chunks each)
    SCS = []
    c = 0
    while c < NCHU:
        n = min(4, NCHU - c)
        SCS.append((c * 128, n))
        c += n
    NEU_T = WPC * EU
    NED_T = WPC * EW

    nc = bacc.Bacc("TRN2", target_bir_lowering=False, debug=False,
                   num_devices=N_CORES)

    def din(name, shape, dt):
        return nc.dram_tensor(name, shape, dt, kind="ExternalInput").ap()

    hT0 = din("hT0", [HID, WPC * APW], F16)
    Gs = din("Gs", [APW, NED_T], F16)        # gather one-hot * C
    Ss = din("Ss", [128, NCHD * WPC * 128], F8)   # scatter one-hot
    ATs = din("ATs", [NG, NEU_T], F16)       # gaussians^T (undirected)
    w1s = din("w1s", [NG, L_INT * HID], F16)
    w2s = din("w2s", [HID, L_INT * HID], F16)
    b1s = din("b1s", [HID, L_INT], F32)
    b2ts = din("b2ts", [1, L_INT * HID], F16)
    cf1s = din("cf1s", [HID, L_INT * HID], F16)
    cf2s = din("cf2s", [HID, L_INT * HID], F16)
    lins = din("lins", [HID, L_INT * HID], F16)
    cf2bs = din("cf2bs", [HID, L_INT], F32)
    linbps = din("linbps", [HID, L_INT], F32)
    out1w = din("out1w", [HID, HID // 2], F16)
    out1b = din("out1b", [HID // 2, 1], F32)
    out2w = din("out2w", [HID // 2, 1], F16)
    fbias = din("fbias", [1, 1], F32)
    out = nc.dram_tensor("out", [1, WPC * MPW], F32, kind="ExternalOutput").ap()

    with tile.TileContext(nc) as tc:
        with ExitStack() as ctx:
            const = ctx.enter_context(tc.tile_pool(name="const", bufs=1))
            slab = ctx.enter_context(tc.tile_pool(name="slab", bufs=1))
            work = ctx.enter_context(tc.tile_pool(name="work", bufs=4))
            nwork = ctx.enter_context(tc.tile_pool(name="nwork", bufs=2))
            psb = ctx.enter_context(tc.tile_pool(name="psb", bufs=1,
                                                 space="PSUM"))
            psx = ctx.enter_context(tc.tile_pool(name="psx", bufs=2,
                                                 space="PSUM"))
            psw = ctx.enter_context(tc.tile_pool(name="psw", bufs=2,
                                                 space="PSUM"))
            psa = ctx.enter_context(tc.tile_pool(name="psa", bufs=2,
                                                 space="PSUM"))

            def load(name, shape, dt, src):
                t = const.tile(shape, dt, tag=name, name=name)
                nc.sync.dma_start(t[:], src)
                return t

            # weights first in the DMA queue: compute can start as soon as
            # window 0's slabs land behind them
            hTa = slab.tile([HID, WPC * APW], F16, tag="hTa")
            nc.sync.dma_start(hTa[:], hT0[:])
            hTb = slab.tile([HID, WPC * APW], F16, tag="hTb")

            w1a = load("w1a", [NG, L_INT * HID], F16, w1s[:])
            w2a = load("w2a", [HID, L_INT * HID], F16, w2s[:])
            b1a = load("b1a", [HID, L_INT], F32, b1s[:])
            b2a = load("b2a", [1, L_INT * HID], F16, b2ts[:])
            cf1a = load("cf1a", [HID, L_INT * HID], F16, cf1s[:])
            cf2a = load("cf2a", [HID, L_INT * HID], F16, cf2s[:])
            lina = load("lina", [HID, L_INT * HID], F16, lins[:])
            cf2ba = load("cf2ba", [HID, L_INT], F32, cf2bs[:])
            linbpa = load("linbpa", [HID, L_INT], F32, linbps[:])
            hs = lambda l: slice(l * HID, (l + 1) * HID)
            w1t = [w1a[:, hs(l)] for l in range(L_INT)]
            w2t = [w2a[:, hs(l)] for l in range(L_INT)]
            b1t = [b1a[:, l:l + 1] for l in range(L_INT)]
            b2t = [b2a[:, hs(l)] for l in range(L_INT)]
            cf1t = [cf1a[:, hs(l)] for l in range(L_INT)]
            cf2t = [cf2a[:, hs(l)] for l in range(L_INT)]
            lint = [lina[:, hs(l)] for l in range(L_INT)]
            cf2bt = [cf2ba[:, l:l + 1] for l in range(L_INT)]
            linbpt = [linbpa[:, l:l + 1] for l in range(L_INT)]
            o1wt = load("o1w", [HID, HID // 2], F16, out1w[:])
            o1bt = load("o1b", [HID // 2, 1], F32, out1b[:])
            o2wt = load("o2w", [HID // 2, 1], F16, out2w[:])
            fbt = load("fb", [1, 1], F32, fbias[:])
            ones1 = const.tile([1, HID], F16, tag="ones1")
            nc.gpsimd.memset(ones1[:], 1.0)

            # slabs streamed per window in use order
            Gt = slab.tile([APW, NED_T], F16, tag="G")
            St = slab.tile([128, NCHD * WPC * 128], F8, tag="S")
            At = slab.tile([NG, NEU_T], F16, tag="A")
            for w in range(WPC):
                nc.sync.dma_start(At[:, w * EU:(w + 1) * EU],
                                  ATs[:, w * EU:(w + 1) * EU])
                nc.sync.dma_start(Gt[:, w * EW:(w + 1) * EW],
                                  Gs[:, w * EW:(w + 1) * EW])
                sl = slice(w * NCHD * 128, (w + 1) * NCHD * 128)
                nc.sync.dma_start(St[:, sl], Ss[:, sl])

            xall = slab.tile([APW, WPC * HID], F16, tag="xall")
            hcur, hnext = hTa, hTb
            for l in range(L_INT):
                # x = h @ cf1, emitted per-window (2-window lookahead) so the
                # staging copies interleave with the edge pipeline
                def emit_cf1(w):
                    wsl = slice(w * APW, (w + 1) * APW)
                    xw_ps = psb.tile([APW, EU], F32, tag="t1",
                                     name=f"xw_{l}_{w}")
                    nc.tensor.matmul(xw_ps[:, 0:HID], hcur[:, wsl],
                                     cf1t[l][:], start=True, stop=True)
                    nc.scalar.copy(xall[:, w * HID:(w + 1) * HID],
                                   xw_ps[:, 0:HID])

                emit_cf1(0)
                emit_cf1(1)

                # 4-deep software pipeline over undirected superchunks:
                # F(i): mlp1+exp (+window-wide Ln at window end)
                # P(i-2): mlp2-flips + both directed gathers
                # D(i-3): W copy to fp16 + two 2x-mode TTs
                # B(i-4): scatters; node stage per 4-window group
                scl = [(w, eoff, nck) for w in range(WPC)
                       for (eoff, nck) in SCS]
                nsc = len(scl)
                npw = len(SCS)               # superchunks per window
                st = [dict() for _ in range(nsc)]
                t1s = {}
                aggps = {}
                aggsb = {}

                def front(i):
                    w, eoff, nck = scl[i]
                    if eoff == 0 and w + 2 < WPC:
                        emit_cf1(w + 2)
                    scw = nck * 128
                    base = w * EU + eoff
                    if eoff == 0:
                        t1s[w] = psb.tile([HID, EU], F32, tag="t1",
                                          name=f"t1_{l}_{w}")
                    t1_ps = t1s[w]
                    nc.tensor.matmul(t1_ps[:, eoff:eoff + scw], w1t[l][:],
                                     At[:, base:base + scw],
                                     start=True, stop=True)
                    if eoff + scw >= EU:
                        esb = work.tile([HID, EU], F16, tag="esb",
                                        name=f"esb_{l}_{w}")
                        nc.scalar.activation(esb[:], t1_ps[:],
                                             AF.Exp, bias=b1t[l][:],
                                             scale=1.0)
                        ssp1 = work.tile([HID, EU], F16, tag="ssp1",
                                         name=f"ssp1_{l}_{w}")
                        nc.scalar.activation(ssp1[:], esb[:],
                                             AF.Ln, bias=1.0, scale=1.0)
                        for j in range(i - npw + 1, i + 1):
                            st[j]["ssp1"] = ssp1

                def pmid(i):
                    w, eoff, nck = scl[i]
                    ssp1 = st[i]["ssp1"]
                    w_ps = psw.tile([128, 512], F32, tag="wps",
                                    name=f"wps_{l}_{i}")
                    xsf_ps = psx.tile([128, 512], F32, tag="xs",
                                      name=f"xsf_{l}_{i}")
                    xsb_ps = psx.tile([128, 512], F32, tag="xs",
                                      name=f"xsb_{l}_{i}")
                    xsl = xall[:, w * HID:(w + 1) * HID]
                    for k in range(nck):
                        ksl = slice(k * 128, (k + 1) * 128)
                        usl = slice(eoff + k * 128, eoff + (k + 1) * 128)
                        nc.tensor.matmul(w_ps[:, ksl], ones1[:], b2t[l][:],
                                         start=True, stop=False)
                        nc.tensor.matmul(w_ps[:, ksl], ssp1[:, usl],
                                         w2t[l][:], start=False, stop=True)
                        gf = w * EW + eoff + k * 128
                        gb = w * EW + EU + eoff + k * 128
                        nc.tensor.matmul(xsf_ps[:, ksl], Gt[:, gf:gf + 128],
                                         xsl, start=True, stop=True)
                        nc.tensor.matmul(xsb_ps[:, ksl], Gt[:, gb:gb + 128],
                                         xsl, start=True, stop=True)
                    scw = nck * 128
                    w_sb = work.tile([128, 512], F16, tag="w_sb",
                                     name=f"wsb_{l}_{i}")
                    if i % 2 == 0:
                        nc.scalar.copy(w_sb[:, 0:scw], w_ps[:, 0:scw])
                    else:
                        nc.vector.tensor_copy(w_sb[:, 0:scw], w_ps[:, 0:scw])
                    st[i]["w_sb"] = w_sb
                    st[i]["xsf_ps"] = xsf_ps
                    st[i]["xsb_ps"] = xsb_ps

                def dmid(i):
                    w, eoff, nck = scl[i]
                    scw = nck * 128
                    w_sb = st[i]["w_sb"]
                    msgf = work.tile([128, 512], F16, tag="msgf",
                                     name=f"msgf_{l}_{i}")
                    nc.vector.tensor_tensor(msgf[:, 0:scw],
                                            st[i]["xsf_ps"][:, 0:scw],
                                            w_sb[:, 0:scw], ALU.mult)
                    msgb = work.tile([128, 512], F16, tag="msgb",
                                     name=f"msgb_{l}_{i}")
                    nc.vector.tensor_tensor(msgb[:, 0:scw],
                                            st[i]["xsb_ps"][:, 0:scw],
                                            w_sb[:, 0:scw], ALU.mult)
                    st[i]["msgf"] = msgf
                    st[i]["msgb"] = msgb

                def node(w, agg_ps):
                    # node stage batched over groups of 4 windows
                    g = w // 4
                    if w % 4 != 3:
                        return
                    gs = slice(g * 512, (g + 1) * 512)
                    aggT = nwork.tile([HID, 512], F16, tag="aggT",
                                      name=f"aggT_{l}_{g}")
                    nc.scalar.copy(aggT[:], agg_ps[:])
                    v1_ps = psb.tile([HID, EU], F32, tag="t1",
                                     name=f"v1_{l}_{g}")[:, 0:512]
                    nc.tensor.matmul(v1_ps[:], cf2t[l][:], aggT[:],
                                     start=True, stop=True)
                    e2 = nwork.tile([HID, 512], F16, tag="e2",
                                    name=f"e2_{l}_{g}")
                    nc.scalar.activation(e2[:], v1_ps[:], AF.Exp,
                                         bias=cf2bt[l][:], scale=1.0)
                    v2 = nwork.tile([HID, 512], F16, tag="v2",
                                    name=f"v2_{l}_{g}")
                    nc.scalar.activation(v2[:], e2[:], AF.Ln, bias=1.0,
                                         scale=1.0)
                    v3_ps = psw.tile([HID, 512], F32, tag="wps",
                                     name=f"v3_{l}_{g}")
                    nc.tensor.matmul(v3_ps[:], lint[l][:], v2[:],
                                     start=True, stop=True)
                    nc.vector.scalar_tensor_tensor(
                        hnext[:, gs], v3_ps[:], linbpt[l][:], hcur[:, gs],
                        ALU.add, ALU.add)

                def back(i):
                    w, eoff, nck = scl[i]
                    g = w // 4
                    if w % 4 == 0 and eoff == 0:
                        aggps[g] = psa.tile([HID, 512], F32, tag="agg",
                                            name=f"agg_{l}_{g}")
                    agg_ps = aggps[g]
                    asl = slice((w % 4) * APW, (w % 4 + 1) * APW)
                    for k in range(nck):
                        ku = (eoff // 128) + k
                        ksl = slice(k * 128, (k + 1) * 128)
                        for d, msg in ((0, st[i]["msgf"]),
                                       (1, st[i]["msgb"])):
                            cg = w * NCHD + d * NCHU + ku
                            ssl = slice(cg * 128, (cg + 1) * 128)
                            nc.tensor.matmul(
                                agg_ps[:, asl], msg[:, ksl], St[:, ssl],
                                start=(eoff == 0 and k == 0 and d == 0),
                                stop=(ku == NCHU - 1 and d == 1))
                    st[i].clear()
                    if eoff + nck * 128 >= EU:
                        node(w, agg_ps)

                for i in range(nsc + 4):
                    if i < nsc:
                        front(i)
                    if 0 <= i - 2 < nsc:
                        pmid(i - 2)
                    if 0 <= i - 3 < nsc:
                        dmid(i - 3)
                    if 0 <= i - 4 < nsc:
                        back(i - 4)
                hcur, hnext = hnext, hcur

            # output head
            outrow = nwork.tile([1, WPC * MPW], F32, tag="outrow")
            for g in range(4):
                asl = slice(g * 512, (g + 1) * 512)
                o1_ps = psb.tile([HID, EU], F32, tag="t1",
                                 name=f"o1_{g}")[0:HID // 2, 0:512]
                nc.tensor.matmul(o1_ps[:], o1wt[:], hcur[:, asl],
                                 start=True, stop=True)
                e3 = work.tile([HID // 2, 512], F16, tag="esb3",
                               name=f"e3_{g}")
                nc.scalar.activation(e3[:], o1_ps[:], AF.Exp,
                                     bias=o1bt[:], scale=1.0)
                o1sb = work.tile([HID // 2, 512], F16, tag="ssp1o",
                                 name=f"o1sb_{g}")
                nc.scalar.activation(o1sb[:], e3[:], AF.Ln, bias=1.0,
                                     scale=1.0)
                o2_ps = psw.tile([1, 512], F32, tag="wps", name=f"o2_{g}")
                nc.tensor.matmul(o2_ps[:], o2wt[:], o1sb[:],
                                 start=True, stop=True)
                red = o2_ps[0:1, 0:512].rearrange("p (m a) -> p m a",
                                                  m=16, a=32)
                nc.vector.tensor_reduce(outrow[0:1, g * 16:(g + 1) * 16],
                                        red, AX.X, ALU.add)
            outfin = nwork.tile([1, WPC * MPW], F32, tag="outfin")
            nc.scalar.activation(outfin[:], outrow[:], AF.Identity,
                                 bias=fbt[:], scale=1.0)
            nc.sync.dma_start(out[:], outfin[:])

    nc.compile()

    # The table-reorder patch above changes the indices the table-load pass
    # assigns, but walrus interprets act_func_set_id as an index into the
    # ORIGINAL act_info.json order. Remap ids back.
    orig_keys = list(_orig_gat("gen3").keys()) if "_orig_gat" in dir() else None
    patched_keys = list(bacc.get_activation_tables("gen3").keys())
    true_idx = {k: i for i, k in
                enumerate(hw_specs.get_activation_tables("gen3").keys())}
    remap = {i: true_idx[k] for i, k in enumerate(patched_keys)}
    for b in nc.main_func.blocks:
        for ins in b.instructions:
            if type(ins).__name__ == "InstLoadActFuncSet":
                ins.act_func_set_id = remap[ins.act_func_set_id]
    return nc


# ----------------------------------------------------------------------------
# Host-side prep + execution
# ----------------------------------------------------------------------------
def _prepare(inputs):
    import concourse.mybir as mybir

    inp = {k: np.asarray(v) for k, v in inputs.items()}
    z = inp["z"].astype(np.int64)
    pos = inp["pos"].astype(np.float64)
    edge_index = inp["edge_index"].astype(np.int64)
    emb = inp["emb"].astype(np.float32)

    src, dst = edge_index[0], edge_index[1]
    mol_of_edge = dst // APM
    mol_cnt = np.bincount(mol_of_edge, minlength=N_MOL)

    # balance molecules -> 128 windows of 4 -> 8 cores of 16 windows
    order = np.argsort(-mol_cnt, kind="stable")
    win_load = np.zeros(N_CORES * WPC, np.int64)
    win_fill = np.zeros(N_CORES * WPC, np.int64)
    win_mols = [[] for _ in range(N_CORES * WPC)]
    for m in order:
        cand = np.flatnonzero(win_fill < MPW)
        wsel = cand[np.argmin(win_load[cand])]
        win_load[wsel] += mol_cnt[m]
        win_fill[wsel] += 1
        win_mols[wsel].append(int(m))
    worder = np.argsort(-win_load, kind="stable")
    core_load = np.zeros(N_CORES, np.int64)
    core_wins = [[] for _ in range(N_CORES)]
    for wsel in worder:
        cand = [c for c in range(N_CORES) if len(core_wins[c]) < WPC]
        csel = min(cand, key=lambda c: core_load[c])
        core_load[csel] += win_load[wsel]
        core_wins[csel].append(int(wsel))

    # undirected capacity per window (directed loads are even)
    EU = int(np.ceil(win_load.max() / 256.0)) * 128
    NCHU = EU // 128
    NCHD = 2 * NCHU
    EW = 2 * EU
    NEU_T = WPC * EU
    NED_T = WPC * EW

    # undirected edges (src < dst); every edge has its reverse
    und_mask = src < dst
    usrc, udst = src[und_mask], dst[und_mask]
    d_u = np.sqrt(((pos[usrc] - pos[udst]) ** 2).sum(-1))
    C_u = 0.5 * (np.cos(d_u * math.pi / CUT) + 1.0)
    offs = np.linspace(0.0, CUT, NG)
    coeff = -0.5 / (CUT / (NG - 1)) ** 2
    umol = udst // APM
    ue_order = np.argsort(umol, kind="stable")
    umol_start = np.searchsorted(umol[ue_order], np.arange(N_MOL + 1))

    mlp_w1 = inp["mlp_w1"].astype(np.float32)
    mlp_b1 = inp["mlp_b1"].astype(np.float32)
    mlp_w2 = inp["mlp_w2"].astype(np.float32)
    mlp_b2 = inp["mlp_b2"].astype(np.float32)
    cf1_w = inp["cf1_w"].astype(np.float32)
    cf2_w = inp["cf2_w"].astype(np.float32)
    cf2_b = inp["cf2_b"].astype(np.float32)
    lin_w = inp["lin_w"].astype(np.float32)
    lin_b = inp["lin_b"].astype(np.float32)
    out1_w = inp["out1_w"].astype(np.float32)
    out1_b = inp["out1_b"].astype(np.float32)
    out2_w = inp["out2_w"].astype(np.float32)
    out2_b = inp["out2_b"].astype(np.float32)

    b2p = mlp_b2 - LN2 * mlp_w2.sum(axis=1)
    linbp = lin_b - LN2 * lin_w.sum(axis=1)
    fb = 32.0 * float(out2_b[0] - LN2 * out2_w.sum())

    f8np = mybir.dt.np(mybir.dt.float8e4)

    in_maps = []
    mol_slot = np.zeros((N_MOL, 2), np.int64)
    for c in range(N_CORES):
        atom_ids = np.empty(WPC * APW, np.int64)
        G_sl = np.zeros((APW, NED_T), np.float16)
        S_sl = np.zeros((128, NCHD * WPC * 128), f8np)
        A_sl = np.zeros((NG, NEU_T), np.float16)
        for wi, wsel in enumerate(core_wins[c]):
            mols = win_mols[wsel]
            for si, m in enumerate(mols):
                atom_ids[wi * APW + si * APM:wi * APW + (si + 1) * APM] = \
                    m * APM + np.arange(APM)
                mol_slot[m] = (c, wi * MPW + si)
            eids = np.concatenate([ue_order[umol_start[m]:umol_start[m + 1]]
                                   for m in mols])
            ne = len(eids)
            assert ne <= EU, (ne, EU)
            loc = {m: si for si, m in enumerate(mols)}
            aml = np.array([loc[m] for m in (usrc[eids] // APM)])
            a_loc = aml * APM + (usrc[eids] % APM)
            b_loc = aml * APM + (udst[eids] % APM)
            cwin = C_u[eids].astype(np.float16)
            dwin = d_u[eids]
            u = np.arange(ne)
            # gather slab: fwd (src=a) at w*EW+u, bwd (src=b) at w*EW+EU+u
            G_sl[a_loc, wi * EW + u] = cwin
            G_sl[b_loc, wi * EW + EU + u] = cwin
            A_sl[:, wi * EU + u] = np.exp(
                coeff * (dwin[None, :] - offs[:, None]) ** 2
            ).astype(np.float16)
            # scatter slab: chunk cg = w*NCHD + dir*NCHU + ku
            ku = u // 128
            cgf = wi * NCHD + ku
            cgb = wi * NCHD + NCHU + ku
            S_sl[u % 128, cgf * 128 + b_loc] = 1.0   # fwd: dst = b
            S_sl[u % 128, cgb * 128 + a_loc] = 1.0   # bwd: dst = a
        h0 = emb[z[atom_ids]]
        m = {
            "hT0": np.ascontiguousarray(h0.T).astype(np.float16),
            "Gs": G_sl,
            "Ss": S_sl,
            "ATs": A_sl,
            "w1s": np.ascontiguousarray(
                mlp_w1.transpose(1, 0, 2).reshape(NG, -1)).astype(np.float16),
            "w2s": np.ascontiguousarray(
                mlp_w2.transpose(1, 0, 2).reshape(HID, -1)).astype(np.float16),
            "b1s": np.ascontiguousarray(mlp_b1.T),
            "b2ts": b2p.reshape(1, -1).astype(np.float16),
            "cf1s": np.ascontiguousarray(
                cf1_w.transpose(1, 0, 2).reshape(HID, -1)).astype(np.float16),
            "cf2s": np.ascontiguousarray(
                cf2_w.transpose(1, 0, 2).reshape(HID, -1)).astype(np.float16),
            "lins": np.ascontiguousarray(
                lin_w.transpose(1, 0, 2).reshape(HID, -1)).astype(np.float16),
            "cf2bs": np.ascontiguousarray(cf2_b.T),
            "linbps": np.ascontiguousarray(linbp.T),
            "out1w": out1_w.astype(np.float16),
            "out1b": out1_b[:, None],
            "out2w": out2_w.astype(np.float16),
            "fbias": np.array([[fb]], np.float32),
        }
        in_maps.append(m)

    return in_maps, mol_slot, EU


def kernel(**inputs):
    from concourse.bass_utils import run_bass_kernel_spmd

    in_maps, mol_slot, EU = _prepare(inputs)
    if EU not in _PROG_CACHE:
        _PROG_CACHE[EU] = _build_program(EU)
    nc = _PROG_CACHE[EU]

    res = run_bass_kernel_spmd(nc, in_maps, core_ids=list(range(N_CORES)))

    out = np.zeros((N_MOL, 1), np.float32)
    for mol in range(N_MOL):
        c, slot = mol_slot[mol]
        out[mol, 0] = res.results[c]["out"][0, slot]
    return out


def measure_hw_time(inputs, iters=30):
    """Time the jitted 8-core executable with device-resident inputs.

    Returns (min_ns, all_ns). Includes PJRT/axon dispatch overhead but big
    inputs stay on device, so deltas between kernel versions are reliable.
    """
    import time
    import jax
    import concourse.mybir as mybir
    from jax.sharding import Mesh, PartitionSpec, NamedSharding
    from jax.experimental.shard_map import shard_map
    from concourse import bass2jax

    in_maps, mol_slot, EU = _prepare(inputs)
    if EU not in _PROG_CACHE:
        _PROG_CACHE[EU] = _build_program(EU)
    nc = _PROG_CACHE[EU]
    bass2jax.install_neuronx_cc_hook()

    pname = nc.partition_id_tensor.name if nc.partition_id_tensor else None
    in_names, out_names, out_avals, zero_outs = [], [], [], []
    for alloc in nc.m.functions[0].allocations:
        if not isinstance(alloc, mybir.MemoryLocationSet):
            continue
        name = alloc.memorylocations[0].name
        if alloc.kind == "ExternalInput":
            if name != pname:
                in_names.append(name)
        elif alloc.kind == "ExternalOutput":
            out_names.append(name)
            shape = tuple(alloc.tensor_shape)
            dtype = mybir.dt.np(alloc.dtype)
            out_avals.append(jax.core.ShapedArray(shape, dtype))
            zero_outs.append(np.zeros(shape, dtype))
    n_params = len(in_names)
    n_outs = len(out_avals)
    all_names = in_names + out_names
    if pname is not None:
        all_names = all_names + [pname]

    def _body(*args):
        operands = list(args)
        if pname is not None:
            operands.append(bass2jax.partition_id_tensor())
        outs = bass2jax._bass_exec_p.bind(
            *operands,
            out_avals=tuple(out_avals),
            in_names=tuple(all_names),
            out_names=tuple(out_names),
            lowering_input_output_aliases=(),
            sim_require_finite=True,
            sim_require_nnan=True,
            nc=nc,
        )
        return tuple(outs)

    devices = jax.devices()[:N_CORES]
    mesh = Mesh(np.asarray(devices), ("core",))
    donate = tuple(range(n_params, n_params + n_outs))
    f = jax.jit(
        shard_map(_body, mesh=mesh,
                  in_specs=(PartitionSpec("core"),) * (n_params + n_outs),
                  out_specs=(PartitionSpec("core"),) * n_outs,
                  check_rep=False),
        donate_argnums=donate, keep_unused=True)

    concat_in = [
        np.concatenate([np.asarray(in_maps[c][nm]) for c in range(N_CORES)],
                       axis=0)
        for nm in in_names
    ]
    sh = NamedSharding(mesh, PartitionSpec("core"))
    dev_in = [jax.device_put(a, sh) for a in concat_in]

    def zouts():
        return [jax.device_put(np.concatenate([z] * N_CORES, axis=0), sh)
                for z in zero_outs]

    r = f(*dev_in, *zouts())
    jax.block_until_ready(r)
    times = []
    for _ in range(iters):
        zo = zouts()
        jax.block_until_ready(zo)
        t0 = time.perf_counter_ns()
        r = f(*dev_in, *zo)
        jax.block_until_ready(r)
        times.append(time.perf_counter_ns() - t0)
    return min(times), times



# revision 42
# speedup vs baseline: 1.0980x; 1.0980x over previous
"""Trainium2 Bass kernel for CustomSchNet (nn_CustomSchNet_43456479101225).

Strategy (graph-level data parallel, 8 cores):
  - 512 molecules load-balanced into 128 windows of 4 molecules (128 atoms),
    16 windows per core.
  - The radius graph is symmetric and the filter W depends only on the
    distance, so the edge-filter MLP + ShiftedSoftplus run once per
    UNDIRECTED edge; each W tile feeds the two directed messages.
  - Filter MLP runs with channels on partitions / edges on free dim
    (weights stationary); the flip to edges-on-partitions is free by using
    the ssp output as the stationary operand of the second MLP matmul.
  - Gather/scatter are one-hot matmuls against host-built fp16/fp8 slabs
    (cosine cutoff folded into the gather slab); messages are fp16 2x-mode
    DVE multiplies.
  - ShiftedSoftplus = Ln(1 + Exp(x)); -ln(2) shifts folded into downstream
    biases on host. Emission order is software-pipelined (4 stages) so each
    cross-engine dependency has superchunks of slack.
"""

import math
import numpy as np

HID = 128
NG = 50
CUT = 6.0
L_INT = 3
APM = 32
N_MOL = 512
N_ATOMS = N_MOL * APM
N_CORES = 8
WPC = 16            # windows per core
APW = 128           # atoms per window (4 molecules)
MPW = 4             # molecules per window
LN2 = math.log(2.0)

_PROG_CACHE = {}


# ----------------------------------------------------------------------------
# Device program
# ----------------------------------------------------------------------------
def _build_program(EU):
    """Build + compile the per-core program. EU = padded undirected
    edges/window (multiple of 128); directed EW = 2*EU."""
    import concourse.bacc as bacc
    import concourse.tile as tile
    import concourse.mybir as mybir
    from contextlib import ExitStack

    # Exp and Ln live in different activation-function tables by default, so
    # the table-load pass would ping-pong 1.3us loads between ssp passes.
    # Reorder so the table holding BOTH (natural_log_exp_and_others) wins.
    import concourse.hw_specs as hw_specs
    if not getattr(hw_specs, "_schnet_tbl_patch", False):
        _orig_gat = hw_specs.get_activation_tables

        def _gat(arch):
            t = dict(_orig_gat(arch))
            key = "natural_log_exp_and_others"
            if key in t:
                t = {key: t[key], **{k: v for k, v in t.items() if k != key}}
            return t

        hw_specs._schnet_tbl_patch = True
        bacc.get_activation_tables = _gat

    F32 = mybir.dt.float32
    F16 = mybir.dt.float16
    F8 = mybir.dt.float8e4
    AF = mybir.ActivationFunctionType
    ALU = mybir.AluOpType
    AX = mybir.AxisListType

    EW = 2 * EU
    NCHU = EU // 128                 # undirected chunks per window
    NCHD = 2 * NCHU                  # directed chunks per window
    # undirected superchunks (<=4 chunks each)
    SCS = []
    c = 0
    while c < NCHU:
        n = min(4, NCHU - c)
        SCS.append((c * 128, n))
        c += n
    NEU_T = WPC * EU
    NED_T = WPC * EW

    nc = bacc.Bacc("TRN2", target_bir_lowering=False, debug=False,
                   num_devices=N_CORES)

    def din(name, shape, dt):
        return nc.dram_tensor(name, shape, dt, kind="ExternalInput").ap()

    hT0 = din("hT0", [HID, WPC * APW], F16)
    Gs = din("Gs", [APW, NED_T], F16)        # gather one-hot * C
    Ss = din("Ss", [128, NCHD * WPC * 128], F8)   # scatter one-hot
    ATs = din("ATs", [NG, NEU_T], F16)       # gaussians^T (undirected)
    w1s = din("w1s", [NG, L_INT * HID], F16)
    w2s = din("w2s", [HID, L_INT * HID], F16)
    b1s = din("b1s", [HID, L_INT], F32)
    b2ts = din("b2ts", [1, L_INT * HID], F16)
    cf1s = din("cf1s", [HID, L_INT * HID], F16)
    cf2s = din("cf2s", [HID, L_INT * HID], F16)
    lins = din("lins", [HID, L_INT * HID], F16)
    cf2bs = din("cf2bs", [HID, L_INT], F32)
    linbps = din("linbps", [HID, L_INT], F32)
    out1w = din("out1w", [HID, HID // 2], F16)
    out1b = din("out1b", [HID // 2, 1], F32)
    out2w = din("out2w", [HID // 2, 1], F16)
    fbias = din("fbias", [1, 1], F32)
    out = nc.dram_tensor("out", [1, WPC * MPW], F32, kind="ExternalOutput").ap()

    with tile.TileContext(nc) as tc:
        with ExitStack() as ctx:
            const = ctx.enter_context(tc.tile_pool(name="const", bufs=1))
            slab = ctx.enter_context(tc.tile_pool(name="slab", bufs=1))
            work = ctx.enter_context(tc.tile_pool(name="work", bufs=4))
            nwork = ctx.enter_context(tc.tile_pool(name="nwork", bufs=2))
            psb = ctx.enter_context(tc.tile_pool(name="psb", bufs=2,
                                                 space="PSUM"))
            psw = ctx.enter_context(tc.tile_pool(name="psw", bufs=1,
                                                 space="PSUM"))
            psa = ctx.enter_context(tc.tile_pool(name="psa", bufs=1,
                                                 space="PSUM"))

            def load(name, shape, dt, src):
                t = const.tile(shape, dt, tag=name, name=name)
                nc.sync.dma_start(t[:], src)
                return t

            # weights first in the DMA queue: compute can start as soon as
            # window 0's slabs land behind them
            hTa = slab.tile([HID, WPC * APW], F16, tag="hTa")
            nc.sync.dma_start(hTa[:], hT0[:])
            hTb = slab.tile([HID, WPC * APW], F16, tag="hTb")

            w1a = load("w1a", [NG, L_INT * HID], F16, w1s[:])
            w2a = load("w2a", [HID, L_INT * HID], F16, w2s[:])
            b1a = load("b1a", [HID, L_INT], F32, b1s[:])
            b2a = load("b2a", [1, L_INT * HID], F16, b2ts[:])
            cf1a = load("cf1a", [HID, L_INT * HID], F16, cf1s[:])
            cf2a = load("cf2a", [HID, L_INT * HID], F16, cf2s[:])
            lina = load("lina", [HID, L_INT * HID], F16, lins[:])
            cf2ba = load("cf2ba", [HID, L_INT], F32, cf2bs[:])
            linbpa = load("linbpa", [HID, L_INT], F32, linbps[:])
            hs = lambda l: slice(l * HID, (l + 1) * HID)
            w1t = [w1a[:, hs(l)] for l in range(L_INT)]
            w2t = [w2a[:, hs(l)] for l in range(L_INT)]
            b1t = [b1a[:, l:l + 1] for l in range(L_INT)]
            b2t = [b2a[:, hs(l)] for l in range(L_INT)]
            cf1t = [cf1a[:, hs(l)] for l in range(L_INT)]
            cf2t = [cf2a[:, hs(l)] for l in range(L_INT)]
            lint = [lina[:, hs(l)] for l in range(L_INT)]
            cf2bt = [cf2ba[:, l:l + 1] for l in range(L_INT)]
            linbpt = [linbpa[:, l:l + 1] for l in range(L_INT)]
            o1wt = load("o1w", [HID, HID // 2], F16, out1w[:])
            o1bt = load("o1b", [HID // 2, 1], F32, out1b[:])
            o2wt = load("o2w", [HID // 2, 1], F16, out2w[:])
            fbt = load("fb", [1, 1], F32, fbias[:])
            ones1 = const.tile([1, HID], F16, tag="ones1")
            nc.gpsimd.memset(ones1[:], 1.0)

            # slabs streamed per window in use order
            Gt = slab.tile([APW, NED_T], F16, tag="G")
            St = slab.tile([128, NCHD * WPC * 128], F8, tag="S")
            At = slab.tile([NG, NEU_T], F16, tag="A")
            for w in range(WPC):
                nc.sync.dma_start(At[:, w * EU:(w + 1) * EU],
                                  ATs[:, w * EU:(w + 1) * EU])
                nc.sync.dma_start(Gt[:, w * EW:(w + 1) * EW],
                                  Gs[:, w * EW:(w + 1) * EW])
                sl = slice(w * NCHD * 128, (w + 1) * NCHD * 128)
                nc.sync.dma_start(St[:, sl], Ss[:, sl])

            xall = slab.tile([APW, WPC * HID], F16, tag="xall")
            hcur, hnext = hTa, hTb
            for l in range(L_INT):
                # x = h @ cf1, emitted per-window (2-window lookahead) so the
                # staging copies interleave with the edge pipeline
                def emit_cf1(w):
                    wsl = slice(w * APW, (w + 1) * APW)
                    xw_ps = psb.tile([APW, HID], F32, tag="t1",
                                     name=f"xw_{l}_{w}")
                    nc.tensor.matmul(xw_ps[:], hcur[:, wsl],
                                     cf1t[l][:], start=True, stop=True)
                    nc.scalar.copy(xall[:, w * HID:(w + 1) * HID],
                                   xw_ps[:])

                emit_cf1(0)
                emit_cf1(1)

                # 4-deep software pipeline over undirected superchunks:
                # F(i): mlp1+exp (+window-wide Ln at window end)
                # P(i-2): mlp2-flips + both directed gathers
                # D(i-3): W copy to fp16 + two 2x-mode TTs
                # B(i-4): scatters; node stage per 4-window group
                scl = [(w, eoff, nck) for w in range(WPC)
                       for (eoff, nck) in SCS]
                nsc = len(scl)
                npw = len(SCS)               # superchunks per window
                st = [dict() for _ in range(nsc)]
                t1s = {}
                aggps = {}
                aggsb = {}

                def front(i):
                    w, eoff, nck = scl[i]
                    if eoff == 0 and w + 2 < WPC:
                        emit_cf1(w + 2)
                    scw = nck * 128
                    base = w * EU + eoff
                    t1_ps = psb.tile([HID, 512], F32, tag="t1",
                                     name=f"t1_{l}_{i}")
                    nc.tensor.matmul(t1_ps[:, 0:scw], w1t[l][:],
                                     At[:, base:base + scw],
                                     start=True, stop=True)
                    if eoff == 0:
                        t1s[w] = work.tile([HID, EU], F16, tag="esb",
                                           name=f"esb_{l}_{w}")
                    esb = t1s[w]
                    nc.scalar.activation(esb[:, eoff:eoff + scw],
                                         t1_ps[:, 0:scw],
                                         AF.Exp, bias=b1t[l][:], scale=1.0)
                    if eoff + scw >= EU:
                        ssp1 = work.tile([HID, EU], F16, tag="ssp1",
                                         name=f"ssp1_{l}_{w}")
                        nc.scalar.activation(ssp1[:], esb[:],
                                             AF.Ln, bias=1.0, scale=1.0)
                        for j in range(i - npw + 1, i + 1):
                            st[j]["ssp1"] = ssp1

                def pmid(i):
                    w, eoff, nck = scl[i]
                    ssp1 = st[i]["ssp1"]
                    w_ps = psw.tile([128, 512], F32, tag="wps",
                                    name=f"wps_{l}_{i}")
                    xsf_ps = psb.tile([128, 512], F32, tag="xsf",
                                      name=f"xsf_{l}_{i}")
                    xsb_ps = psb.tile([128, 512], F32, tag="xsb",
                                      name=f"xsb_{l}_{i}")
                    xsl = xall[:, w * HID:(w + 1) * HID]
                    for k in range(nck):
                        ksl = slice(k * 128, (k + 1) * 128)
                        usl = slice(eoff + k * 128, eoff + (k + 1) * 128)
                        nc.tensor.matmul(w_ps[:, ksl], ones1[:], b2t[l][:],
                                         start=True, stop=False)
                        nc.tensor.matmul(w_ps[:, ksl], ssp1[:, usl],
                                         w2t[l][:], start=False, stop=True)
                        gf = w * EW + eoff + k * 128
                        gb = w * EW + EU + eoff + k * 128
                        nc.tensor.matmul(xsf_ps[:, ksl], Gt[:, gf:gf + 128],
                                         xsl, start=True, stop=True)
                        nc.tensor.matmul(xsb_ps[:, ksl], Gt[:, gb:gb + 128],
                                         xsl, start=True, stop=True)
                    scw = nck * 128
                    w_sb = work.tile([128, 512], F16, tag="w_sb",
                                     name=f"wsb_{l}_{i}")
                    if i % 2 == 0:
                        nc.scalar.copy(w_sb[:, 0:scw], w_ps[:, 0:scw])
                    else:
                        nc.vector.tensor_copy(w_sb[:, 0:scw], w_ps[:, 0:scw])
                    st[i]["w_sb"] = w_sb
                    st[i]["xsf_ps"] = xsf_ps
                    st[i]["xsb_ps"] = xsb_ps

                def dmid(i):
                    w, eoff, nck = scl[i]
                    scw = nck * 128
                    w_sb = st[i]["w_sb"]
                    msgf = work.tile([128, 512], F16, tag="msgf",
                                     name=f"msgf_{l}_{i}")
                    nc.vector.tensor_tensor(msgf[:, 0:scw],
                                            st[i]["xsf_ps"][:, 0:scw],
                                            w_sb[:, 0:scw], ALU.mult)
                    msgb = work.tile([128, 512], F16, tag="msgb",
                                     name=f"msgb_{l}_{i}")
                    nc.vector.tensor_tensor(msgb[:, 0:scw],
                                            st[i]["xsb_ps"][:, 0:scw],
                                            w_sb[:, 0:scw], ALU.mult)
                    st[i]["msgf"] = msgf
                    st[i]["msgb"] = msgb

                def node(w, agg_ps):
                    # node stage batched over groups of 4 windows
                    g = w // 4
                    if w % 4 != 3:
                        return
                    gs = slice(g * 512, (g + 1) * 512)
                    aggT = nwork.tile([HID, 512], F16, tag="aggT",
                                      name=f"aggT_{l}_{g}")
                    nc.scalar.copy(aggT[:], agg_ps[:])
                    v1_ps = psb.tile([HID, 512], F32, tag="t1",
                                     name=f"v1_{l}_{g}")
                    nc.tensor.matmul(v1_ps[:], cf2t[l][:], aggT[:],
                                     start=True, stop=True)
                    e2 = nwork.tile([HID, 512], F16, tag="e2",
                                    name=f"e2_{l}_{g}")
                    nc.scalar.activation(e2[:], v1_ps[:], AF.Exp,
                                         bias=cf2bt[l][:], scale=1.0)
                    v2 = nwork.tile([HID, 512], F16, tag="v2",
                                    name=f"v2_{l}_{g}")
                    nc.scalar.activation(v2[:], e2[:], AF.Ln, bias=1.0,
                                         scale=1.0)
                    v3_ps = psw.tile([HID, 512], F32, tag="wps",
                                     name=f"v3_{l}_{g}")
                    nc.tensor.matmul(v3_ps[:], lint[l][:], v2[:],
                                     start=True, stop=True)
                    nc.vector.scalar_tensor_tensor(
                        hnext[:, gs], v3_ps[:], linbpt[l][:], hcur[:, gs],
                        ALU.add, ALU.add)

                def back(i):
                    w, eoff, nck = scl[i]
                    g = w // 4
                    if w % 4 == 0 and eoff == 0:
                        aggps[g] = psa.tile([HID, 512], F32, tag="agg",
                                            name=f"agg_{l}_{g}")
                    agg_ps = aggps[g]
                    asl = slice((w % 4) * APW, (w % 4 + 1) * APW)
                    for k in range(nck):
                        ku = (eoff // 128) + k
                        ksl = slice(k * 128, (k + 1) * 128)
                        for d, msg in ((0, st[i]["msgf"]),
                                       (1, st[i]["msgb"])):
                            cg = w * NCHD + d * NCHU + ku
                            ssl = slice(cg * 128, (cg + 1) * 128)
                            nc.tensor.matmul(
                                agg_ps[:, asl], msg[:, ksl], St[:, ssl],
                                start=(eoff == 0 and k == 0 and d == 0),
                                stop=(ku == NCHU - 1 and d == 1))
                    st[i].clear()
                    if eoff + nck * 128 >= EU:
                        node(w, agg_ps)

                for i in range(nsc + 4):
                    if i < nsc:
                        front(i)
                    if 0 <= i - 2 < nsc:
                        pmid(i - 2)
                    if 0 <= i - 3 < nsc:
                        dmid(i - 3)
                    if 0 <= i - 4 < nsc:
                        back(i - 4)
                hcur, hnext = hnext, hcur

            # output head
            outrow = nwork.tile([1, WPC * MPW], F32, tag="outrow")
            for g in range(4):
                asl = slice(g * 512, (g + 1) * 512)
                o1_ps = psb.tile([HID // 2, 512], F32, tag="t1",
                                 name=f"o1_{g}")
                nc.tensor.matmul(o1_ps[:], o1wt[:], hcur[:, asl],
                                 start=True, stop=True)
                e3 = work.tile([HID // 2, 512], F16, tag="esb3",
                               name=f"e3_{g}")
                nc.scalar.activation(e3[:], o1_ps[:], AF.Exp,
                                     bias=o1bt[:], scale=1.0)
                o1sb = work.tile([HID // 2, 512], F16, tag="ssp1o",
                                 name=f"o1sb_{g}")
                nc.scalar.activation(o1sb[:], e3[:], AF.Ln, bias=1.0,
                                     scale=1.0)
                o2_ps = psw.tile([1, 512], F32, tag="wps", name=f"o2_{g}")
                nc.tensor.matmul(o2_ps[:], o2wt[:], o1sb[:],
                                 start=True, stop=True)
                red = o2_ps[0:1, 0:512].rearrange("p (m a) -> p m a",
                                                  m=16, a=32)
                nc.vector.tensor_reduce(outrow[0:1, g * 16:(g + 1) * 16],
                                        red, AX.X, ALU.add)
            outfin = nwork.tile([1, WPC * MPW], F32, tag="outfin")
            nc.scalar.activation(outfin[:], outrow[:], AF.Identity,
                                 bias=fbt[:], scale=1.0)
            nc.sync.dma_start(out[:], outfin[:])

    nc.compile()

    # The table-reorder patch above changes the indices the table-load pass
    # assigns, but walrus interprets act_func_set_id as an index into the
    # ORIGINAL act_info.json order. Remap ids back.
    orig_keys = list(_orig_gat("gen3").keys()) if "_orig_gat" in dir() else None
    patched_keys = list(bacc.get_activation_tables("gen3").keys())
    true_idx = {k: i for i, k in
                enumerate(hw_specs.get_activation_tables("gen3").keys())}
    remap = {i: true_idx[k] for i, k in enumerate(patched_keys)}
    for b in nc.main_func.blocks:
        for ins in b.instructions:
            if type(ins).__name__ == "InstLoadActFuncSet":
                ins.act_func_set_id = remap[ins.act_func_set_id]
    return nc


# ----------------------------------------------------------------------------
# Host-side prep + execution
# ----------------------------------------------------------------------------
def _prepare(inputs):
    import concourse.mybir as mybir

    inp = {k: np.asarray(v) for k, v in inputs.items()}
    z = inp["z"].astype(np.int64)
    pos = inp["pos"].astype(np.float64)
    edge_index = inp["edge_index"].astype(np.int64)
    emb = inp["emb"].astype(np.float32)

    src, dst = edge_index[0], edge_index[1]
    mol_of_edge = dst // APM
    mol_cnt = np.bincount(mol_of_edge, minlength=N_MOL)

    # balance molecules -> 128 windows of 4 -> 8 cores of 16 windows
    order = np.argsort(-mol_cnt, kind="stable")
    win_load = np.zeros(N_CORES * WPC, np.int64)
    win_fill = np.zeros(N_CORES * WPC, np.int64)
    win_mols = [[] for _ in range(N_CORES * WPC)]
    for m in order:
        cand = np.flatnonzero(win_fill < MPW)
        wsel = cand[np.argmin(win_load[cand])]
        win_load[wsel] += mol_cnt[m]
        win_fill[wsel] += 1
        win_mols[wsel].append(int(m))
    worder = np.argsort(-win_load, kind="stable")
    core_load = np.zeros(N_CORES, np.int64)
    core_wins = [[] for _ in range(N_CORES)]
    for wsel in worder:
        cand = [c for c in range(N_CORES) if len(core_wins[c]) < WPC]
        csel = min(cand, key=lambda c: core_load[c])
        core_load[csel] += win_load[wsel]
        core_wins[csel].append(int(wsel))

    # undirected capacity per window (directed loads are even)
    EU = int(np.ceil(win_load.max() / 256.0)) * 128
    NCHU = EU // 128
    NCHD = 2 * NCHU
    EW = 2 * EU
    NEU_T = WPC * EU
    NED_T = WPC * EW

    # undirected edges (src < dst); every edge has its reverse
    und_mask = src < dst
    usrc, udst = src[und_mask], dst[und_mask]
    d_u = np.sqrt(((pos[usrc] - pos[udst]) ** 2).sum(-1))
    C_u = 0.5 * (np.cos(d_u * math.pi / CUT) + 1.0)
    offs = np.linspace(0.0, CUT, NG)
    coeff = -0.5 / (CUT / (NG - 1)) ** 2
    umol = udst // APM
    ue_order = np.argsort(umol, kind="stable")
    umol_start = np.searchsorted(umol[ue_order], np.arange(N_MOL + 1))

    mlp_w1 = inp["mlp_w1"].astype(np.float32)
    mlp_b1 = inp["mlp_b1"].astype(np.float32)
    mlp_w2 = inp["mlp_w2"].astype(np.float32)
    mlp_b2 = inp["mlp_b2"].astype(np.float32)
    cf1_w = inp["cf1_w"].astype(np.float32)
    cf2_w = inp["cf2_w"].astype(np.float32)
    cf2_b = inp["cf2_b"].astype(np.float32)
    lin_w = inp["lin_w"].astype(np.float32)
    lin_b = inp["lin_b"].astype(np.float32)
    out1_w = inp["out1_w"].astype(np.float32)
    out1_b = inp["out1_b"].astype(np.float32)
    out2_w = inp["out2_w"].astype(np.float32)
    out2_b = inp["out2_b"].astype(np.float32)

    b2p = mlp_b2 - LN2 * mlp_w2.sum(axis=1)
    linbp = lin_b - LN2 * lin_w.sum(axis=1)
    fb = 32.0 * float(out2_b[0] - LN2 * out2_w.sum())

    f8np = mybir.dt.np(mybir.dt.float8e4)

    in_maps = []
    mol_slot = np.zeros((N_MOL, 2), np.int64)
    for c in range(N_CORES):
        atom_ids = np.empty(WPC * APW, np.int64)
        G_sl = np.zeros((APW, NED_T), np.float16)
        S_sl = np.zeros((128, NCHD * WPC * 128), f8np)
        A_sl = np.zeros((NG, NEU_T), np.float16)
        for wi, wsel in enumerate(core_wins[c]):
            mols = win_mols[wsel]
            for si, m in enumerate(mols):
                atom_ids[wi * APW + si * APM:wi * APW + (si + 1) * APM] = \
                    m * APM + np.arange(APM)
                mol_slot[m] = (c, wi * MPW + si)
            eids = np.concatenate([ue_order[umol_start[m]:umol_start[m + 1]]
                                   for m in mols])
            ne = len(eids)
            assert ne <= EU, (ne, EU)
            loc = {m: si for si, m in enumerate(mols)}
            aml = np.array([loc[m] for m in (usrc[eids] // APM)])
            a_loc = aml * APM + (usrc[eids] % APM)
            b_loc = aml * APM + (udst[eids] % APM)
            cwin = C_u[eids].astype(np.float16)
            dwin = d_u[eids]
            u = np.arange(ne)
            # gather slab: fwd (src=a) at w*EW+u, bwd (src=b) at w*EW+EU+u
            G_sl[a_loc, wi * EW + u] = cwin
            G_sl[b_loc, wi * EW + EU + u] = cwin
            A_sl[:, wi * EU + u] = np.exp(
                coeff * (dwin[None, :] - offs[:, None]) ** 2
            ).astype(np.float16)
            # scatter slab: chunk cg = w*NCHD + dir*NCHU + ku
            ku = u // 128
            cgf = wi * NCHD + ku
            cgb = wi * NCHD + NCHU + ku
            S_sl[u % 128, cgf * 128 + b_loc] = 1.0   # fwd: dst = b
            S_sl[u % 128, cgb * 128 + a_loc] = 1.0   # bwd: dst = a
        h0 = emb[z[atom_ids]]
        m = {
            "hT0": np.ascontiguousarray(h0.T).astype(np.float16),
            "Gs": G_sl,
            "Ss": S_sl,
            "ATs": A_sl,
            "w1s": np.ascontiguousarray(
                mlp_w1.transpose(1, 0, 2).reshape(NG, -1)).astype(np.float16),
            "w2s": np.ascontiguousarray(
                mlp_w2.transpose(1, 0, 2).reshape(HID, -1)).astype(np.float16),
            "b1s": np.ascontiguousarray(mlp_b1.T),
            "b2ts": b2p.reshape(1, -1).astype(np.float16),
            "cf1s": np.ascontiguousarray(
                cf1_w.transpose(1, 0, 2).reshape(HID, -1)).astype(np.float16),
            "cf2s": np.ascontiguousarray(
                cf2_w.transpose(1, 0, 2).reshape(HID, -1)).astype(np.float16),
            "lins": np.ascontiguousarray(
                lin_w.transpose(1, 0, 2).reshape(HID, -1)).astype(np.float16),
            "cf2bs": np.ascontiguousarray(cf2_b.T),
            "linbps": np.ascontiguousarray(linbp.T),
            "out1w": out1_w.astype(np.float16),
            "out1b": out1_b[:, None],
            "out2w": out2_w.astype(np.float16),
            "fbias": np.array([[fb]], np.float32),
        }
        in_maps.append(m)

    return in_maps, mol_slot, EU


def kernel(**inputs):
    from concourse.bass_utils import run_bass_kernel_spmd

    in_maps, mol_slot, EU = _prepare(inputs)
    if EU not in _PROG_CACHE:
        _PROG_CACHE[EU] = _build_program(EU)
    nc = _PROG_CACHE[EU]

    res = run_bass_kernel_spmd(nc, in_maps, core_ids=list(range(N_CORES)))

    out = np.zeros((N_MOL, 1), np.float32)
    for mol in range(N_MOL):
        c, slot = mol_slot[mol]
        out[mol, 0] = res.results[c]["out"][0, slot]
    return out


def measure_hw_time(inputs, iters=30):
    """Time the jitted 8-core executable with device-resident inputs.

    Returns (min_ns, all_ns). Includes PJRT/axon dispatch overhead but big
    inputs stay on device, so deltas between kernel versions are reliable.
    """
    import time
    import jax
    import concourse.mybir as mybir
    from jax.sharding import Mesh, PartitionSpec, NamedSharding
    from jax.experimental.shard_map import shard_map
    from concourse import bass2jax

    in_maps, mol_slot, EU = _prepare(inputs)
    if EU not in _PROG_CACHE:
        _PROG_CACHE[EU] = _build_program(EU)
    nc = _PROG_CACHE[EU]
    bass2jax.install_neuronx_cc_hook()

    pname = nc.partition_id_tensor.name if nc.partition_id_tensor else None
    in_names, out_names, out_avals, zero_outs = [], [], [], []
    for alloc in nc.m.functions[0].allocations:
        if not isinstance(alloc, mybir.MemoryLocationSet):
            continue
        name = alloc.memorylocations[0].name
        if alloc.kind == "ExternalInput":
            if name != pname:
                in_names.append(name)
        elif alloc.kind == "ExternalOutput":
            out_names.append(name)
            shape = tuple(alloc.tensor_shape)
            dtype = mybir.dt.np(alloc.dtype)
            out_avals.append(jax.core.ShapedArray(shape, dtype))
            zero_outs.append(np.zeros(shape, dtype))
    n_params = len(in_names)
    n_outs = len(out_avals)
    all_names = in_names + out_names
    if pname is not None:
        all_names = all_names + [pname]

    def _body(*args):
        operands = list(args)
        if pname is not None:
            operands.append(bass2jax.partition_id_tensor())
        outs = bass2jax._bass_exec_p.bind(
            *operands,
            out_avals=tuple(out_avals),
            in_names=tuple(all_names),
            out_names=tuple(out_names),
            lowering_input_output_aliases=(),
            sim_require_finite=True,
            sim_require_nnan=True,
            nc=nc,
        )
        return tuple(outs)

    devices = jax.devices()[:N_CORES]
    mesh = Mesh(np.asarray(devices), ("core",))
    donate = tuple(range(n_params, n_params + n_outs))
    f = jax.jit(
        shard_map(_body, mesh=mesh,
                  in_specs=(PartitionSpec("core"),) * (n_params + n_outs),
                  out_specs=(PartitionSpec("core"),) * n_outs,
                  check_rep=False),
        donate_argnums=donate, keep_unused=True)

    concat_in = [
        np.concatenate([np.asarray(in_maps[c][nm]) for c in range(N_CORES)],
                       axis=0)
        for nm in in_names
    ]
    sh = NamedSharding(mesh, PartitionSpec("core"))
    dev_in = [jax.device_put(a, sh) for a in concat_in]

    def zouts():
        return [jax.device_put(np.concatenate([z] * N_CORES, axis=0), sh)
                for z in zero_outs]

    r = f(*dev_in, *zouts())
    jax.block_until_ready(r)
    times = []
    for _ in range(iters):
        zo = zouts()
        jax.block_until_ready(zo)
        t0 = time.perf_counter_ns()
        r = f(*dev_in, *zo)
        jax.block_until_ready(r)
        times.append(time.perf_counter_ns() - t0)
    return min(times), times

